# revision 33
# baseline (speedup 1.0000x reference)
"""BERT-CRF Viterbi decode on 8 Trainium2 NeuronCores.

Strategy (data-parallel over batch, 8 samples/core):
  Phase A (feats = x @ W + b):
    - DMA x naturally as [128bt, 768] tiles
    - PE-transpose 128x128 chunks -> PSUM, ACT-copy -> SBUF (x^T tiles)
    - f32r PE matmul W_chunk.T @ x^T -> feats^T [6, 512] per sample (PSUM),
      ACT copy (+bias) -> SBUF, SBUF->SBUF DMA into scan layout
      F[(s,c) partition, n*32+tch free]
  Phase B (Viterbi as associative (max,+) scans on DVE):
    - pass1: per-chunk composite tree (span 2,4,8,16,32), leaf pairs use the
      precomputed T2[i,p,k] = trans[i,k]+trans[k,p] trick
    - pass2: fwd/bwd boundary scans across the 16 chunks per sample
    - pass3: alpha_t / beta_t for every t from boundaries + tree composites
    - path_t = first-argmax_n(alpha_t[n] + beta_t[n]); score = max_n(...)
  No sequential backtrace: the optimal path is read off alpha+beta directly.
"""

import numpy as np

B, T, H, K = 64, 512, 768, 6


def _split_multiwaits(nc):
    """Walrus limits most engine instructions to ONE attached sync wait.
    Split extra waits into standalone EventSemaphore instructions on the
    same engine queue (executed in order before the instruction)."""
    import concourse.mybir as mybir

    for f in nc.m.functions:
        for bb in f.blocks:
            newlist = []
            changed = False
            for ins in bb.instructions:
                si = ins.sync_info
                if si is not None and si.on_wait and len(si.on_wait) > 1:
                    waits = list(si.on_wait)
                    for i, w in enumerate(waits[:-1]):
                        ev = mybir.InstEventSemaphore(
                            name=f"{ins.name}_presync{i}", ins=[], outs=[]
                        )
                        ev.engine = ins.engine
                        ev.sync_info = mybir.SyncInfo(on_wait=[w], on_update=[])
                        newlist.append(ev)
                    ins.sync_info = mybir.SyncInfo(
                        on_wait=[waits[-1]], on_update=list(si.on_update or [])
                    )
                    changed = True
                newlist.append(ins)
            if changed:
                bb.instructions = newlist
    return nc
START_IDX, STOP_IDX = 4, 5
NEG = np.float32(-10000.0)
NCORES = 8
S = B // NCORES          # samples per core
NCH = 16                 # chunks per sample
SPAN = T // NCH          # 32 timesteps per chunk
ROWS = S * NCH           # 128 partition rows (s*16 + c)
GRP = 4                  # 128-row subtiles per sample (512 = 4*128)
HC = H // 128            # 6 h-chunks


def build_module(split=True):
    import concourse.bass as bass
    import concourse.mybir as mybir
    from concourse.tile import TileContext

    f32 = mybir.dt.float32
    f32r = mybir.dt.float32r
    i32 = mybir.dt.int32
    ADD = mybir.AluOpType.add
    MAX = mybir.AluOpType.max
    MULT = mybir.AluOpType.mult
    EQ = mybir.AluOpType.is_equal
    X = mybir.AxisListType.X
    COPY = mybir.ActivationFunctionType.Copy

    nc = bass.Bass()

    from concourse.bass import _add_dep_helper

    xs = nc.dram_tensor("xs", [S, T, H], f32, kind="ExternalInput")
    wc_d = nc.dram_tensor("wc", [128, HC * K], f32, kind="ExternalInput")
    bvec_d = nc.dram_tensor("bvec", [K, 1], f32, kind="ExternalInput")
    transR_d = nc.dram_tensor("transR", [128, 36], f32, kind="ExternalInput")
    t2r_d = nc.dram_tensor("t2r", [128, 216], f32, kind="ExternalInput")
    cn_d = nc.dram_tensor("cn", [128, K * SPAN], f32, kind="ExternalInput")
    a0_d = nc.dram_tensor("a0", [128, K], f32, kind="ExternalInput")
    ident_d = nc.dram_tensor("ident", [128, 128], f32, kind="ExternalInput")
    fstage_d = nc.dram_tensor("fstage", [S, K, 512], f32)
    c32stage_d = nc.dram_tensor("c32stage", [ROWS, 36], f32)
    abstage_d = nc.dram_tensor("abstage", [S, NCH * K], f32)
    bbstage_d = nc.dram_tensor("bbstage", [S, NCH * K], f32)
    ps_out = nc.dram_tensor("ps", [S, 1], f32, kind="ExternalOutput")
    path_out = nc.dram_tensor("path", [S, T], i32, kind="ExternalOutput")

    with TileContext(nc) as tc:
        with (
            tc.tile_pool(name="const", bufs=1) as cpool,
            tc.tile_pool(name="xin", bufs=8) as xpool,
            tc.tile_pool(name="xt", bufs=6) as xtpool,
            tc.tile_pool(name="tiny", bufs=48) as tinypool,
            tc.tile_pool(name="fts", bufs=8) as ftspool,
            tc.tile_pool(name="scan", bufs=1) as spool,
            tc.tile_pool(name="scr", bufs=2) as upool,
            tc.tile_pool(name="tp", bufs=3, space="PSUM") as tpsum,
            tc.tile_pool(name="fp", bufs=2, space="PSUM") as fpsum,
            tc.tile_pool(name="wp", bufs=1, space="PSUM") as wpool,
        ):
            # ---- constants ----
            wc_raw = cpool.tile([128, HC * K], f32, tag="wcraw")
            wc_t = cpool.tile([128, HC * K], f32r, tag="wc")
            bvec_t = cpool.tile([K, 1], f32, tag="bvec")
            transR_t = cpool.tile([128, 36], f32, tag="transR")
            t2r_t = cpool.tile([128, 216], f32, tag="t2r")
            cn_t = cpool.tile([128, K * SPAN], f32, tag="cn")
            a0_t = cpool.tile([128, K], f32, tag="a0")
            ident_t = cpool.tile([128, 128], f32, tag="ident")
            for tile, dram in (
                (wc_raw, wc_d), (bvec_t, bvec_d), (transR_t, transR_d),
                (t2r_t, t2r_d), (cn_t, cn_d), (a0_t, a0_d), (ident_t, ident_d),
            ):
                nc.sync.dma_start(tile[:], dram[:])
            # fp32r requires inputs produced by a rounding compute op
            nc.vector.tensor_copy(wc_t[:], wc_raw[:])
            ident2_t = cpool.tile([128, 128], f32r, tag="ident2")
            nc.vector.tensor_copy(ident2_t[:], ident_t[:])
            # early ACT touch of bvec so the first bias-add has one wait
            bscr = cpool.tile([K, 1], f32, tag="bscr")
            nc.scalar.copy(bscr[:], bvec_t[:])

            # F[(s,c) row, n*SPAN + tch]
            F_t = spool.tile([ROWS, K * SPAN], f32, tag="F")

            # PE consumes ident/wc once up-front so later matmuls carry at
            # most one semaphore wait each (walrus S3_LW wait-slot limit).
            warm = wpool.tile([128, 512], f32, tag="warm")
            nc.tensor.transpose(warm[:, 0:128], ident_t[:], ident_t[:])
            nc.tensor.transpose(
                warm[0:HC * K, 128:256].bitcast(f32r), wc_t[:], ident2_t[:]
            )

            # ---- Phase A: feats ----
            # HWDGE DMA triggers / fused matmuls / activations only take ONE
            # sync wait in their ISA structs, so: x loads + re-layouts go via
            # SWDGE (gpsimd, software-dispatched, flexible waits); each xT
            # copy gets a tiny ACT pre-absorber for the PE wait; fts/tiny
            # pools are no-reuse so no slot-release waits land on ACT ops.
            for s in range(S):
                xin = xpool.tile([128, GRP * H], f32, tag="xin")
                src = xs[s].rearrange("(g p) h -> p g h", p=128)
                nc.sync.dma_start(
                    xin[:].rearrange("p (g h) -> p g h", g=GRP), src
                )
                # dummy PE read of the x tile: absorbs its DMA wait so the
                # first real transpose keeps a single wait slot
                dummy = nc.tensor.transpose(
                    warm[0:1, 0:128], xin[:, 0:1], ident_t[:]
                )
                ftp = fpsum.tile([K, 512], f32, tag="ftp")
                for hc in range(HC):
                    pt = tpsum.tile([128, 512], f32, tag="pt")
                    for sub in range(GRP):
                        tp_i = nc.tensor.transpose(
                            pt[:, sub * 128:(sub + 1) * 128],
                            xin[:, sub * H + hc * 128:sub * H + (hc + 1) * 128],
                            ident_t[:],
                        )
                        if hc == 0 and sub == 0:
                            _add_dep_helper(
                                tp_i.ins, dummy.ins, sync=False,
                                reason="keep dummy before first transpose",
                            )
                    tiny = tinypool.tile([1, 4], f32, tag="tiny")
                    nc.scalar.copy(tiny[:], pt[0:1, 0:512:128])
                    xT = xtpool.tile([128, 512], f32r, tag="xt")
                    nc.scalar.copy(xT[:], pt[:])
                    nc.tensor.matmul(
                        ftp[:],
                        wc_t[:, hc * K:(hc + 1) * K],
                        xT[:],
                        start=(hc == 0),
                        stop=(hc == HC - 1),
                    )
                fts = ftspool.tile([K, 512], f32, tag="fts")
                nc.scalar.add(fts[:], ftp[:], bvec_t[:])
                # re-layout via DRAM staging (SBUF->SBUF DMAs get chained
                # by the deadlock workaround and accumulate waits; DRAM APs
                # also allow arbitrary dim order, so one DMA each way)
                nc.sync.dma_start(fstage_d[s], fts[:])
                dstF = F_t[s * NCH:(s + 1) * NCH, :].rearrange(
                    "c (n t) -> c n t", n=K
                )
                srcF = (
                    fstage_d[s].rearrange("n (c t) -> n c t", c=NCH)
                    .transpose([1, 0, 2])
                )
                nc.sync.dma_start(dstF, srcF)
                # DVE touch absorbs this DMA's lane wait before pass 1
                # (read spans from partition 0: engines need base 0/32/64/96)
                ftouch = tinypool.tile([128, 1], f32, tag=f"ftouch{s}")
                nc.vector.tensor_copy(
                    ftouch[0:(s + 1) * NCH, :], F_t[0:(s + 1) * NCH, 0:1]
                )

            # ---- Phase B ----
            # pass 1: chunk-composite tree.  All tiles laid out
            # [(s,c) row, g*36 + i*6 + p] (i = new state, p = chunk-start state)
            C2_t = spool.tile([ROWS, 16 * 36], f32, tag="C2")
            C4_t = spool.tile([ROWS, 8 * 36], f32, tag="C4")
            C8_t = spool.tile([ROWS, 4 * 36], f32, tag="C8")
            C16_t = spool.tile([ROWS, 2 * 36], f32, tag="C16")
            C32_t = spool.tile([ROWS, 36], f32, tag="C32")

            F3 = F_t[:].rearrange("r (n t) -> r n t", n=K)  # [128, 6, 32]

            # L1: C2_g = M_{2g+1} (x) M_{2g} via T2 trick
            # U[(g,ip,k)] = T2[(ip,k)] + feat_{2g}[k]
            U1 = upool.tile([ROWS, 16 * 216], f32, tag="U")
            in1 = (
                t2r_t[:].rearrange("r (ip k) -> r ip k", k=K)
                .unsqueeze(1).broadcast_to([ROWS, 16, 36, K])
            )
            in2 = (
                F3[:, :, 0:SPAN:2].transpose([0, 2, 1])
                .unsqueeze(2).broadcast_to([ROWS, 16, 36, K])
            )
            u1v = U1[:].rearrange("r (g ip k) -> r g ip k", g=16, ip=36)
            nc.vector.tensor_tensor(u1v, in1, in2, op=ADD)
            V1 = upool.tile([ROWS, 16 * 36], f32, tag="V")
            nc.vector.tensor_reduce(
                V1[:].rearrange("r (g ip) -> r g ip", g=16), u1v, axis=X, op=MAX
            )
            # C2 = V1 + feat_{2g+1}[i]  (broadcast over p)
            in2b = (
                F3[:, :, 1:SPAN:2].transpose([0, 2, 1])
                .unsqueeze(3).broadcast_to([ROWS, 16, K, K])
            )
            nc.vector.tensor_tensor(
                C2_t[:].rearrange("r (g i p) -> r g i p", g=16, i=K),
                V1[:].rearrange("r (g i p) -> r g i p", g=16, i=K),
                in2b,
                op=ADD,
            )

            def combine(dst_t, src_t, npairs):
                # dst[g] = src[2g+1] (x) src[2g]
                # (TensorTensor ISA allows max 3 free dims -> one op per pair)
                U = upool.tile([ROWS, npairs * 216], f32, tag="U")
                sv = src_t[:].rearrange("r (g b) -> r g b", b=72)
                uv4 = U[:].rearrange(
                    "r (g i p k) -> r g i p k", g=npairs, i=K, p=K
                )
                for g in range(npairs):
                    a_v = (
                        sv[:, g, 36:72].rearrange("r (i k) -> r i k", i=K)
                        .unsqueeze(2).broadcast_to([ROWS, K, K, K])
                    )
                    b_v = (
                        sv[:, g, 0:36].rearrange("r (k p) -> r k p", k=K)
                        .transpose([0, 2, 1])
                        .unsqueeze(1).broadcast_to([ROWS, K, K, K])
                    )
                    nc.vector.tensor_tensor(uv4[:, g], a_v, b_v, op=ADD)
                    nc.vector.tensor_reduce(
                        dst_t[:, g * 36:(g + 1) * 36].rearrange(
                            "r (i p) -> r i p", i=K
                        ),
                        uv4[:, g], axis=X, op=MAX,
                    )

            combine(C4_t, C2_t, 8)
            combine(C8_t, C4_t, 4)
            combine(C16_t, C8_t, 2)
            combine(C32_t, C16_t, 1)

            # pass 2: boundary scans (rows 0..7 = samples)
            c32l = spool.tile([S, NCH * 36], f32, tag="c32l")
            nc.sync.dma_start(c32stage_d[:], C32_t[:])
            nc.sync.dma_start(
                c32l[:].rearrange("s (c f) -> s c f", c=NCH),
                c32stage_d[:].rearrange("(s c) f -> s c f", c=NCH),
            )
            ABf = spool.tile([S, NCH * K], f32, tag="ABf")
            BBw = spool.tile([S, NCH * K], f32, tag="BBw")
            nc.vector.tensor_copy(ABf[:, 0:K], a0_t[0:S, :])
            nc.vector.tensor_copy(
                BBw[:, (NCH - 1) * K:NCH * K],
                transR_t[0:S, STOP_IDX * K:(STOP_IDX + 1) * K],
            )
            for c in range(NCH - 1):
                Ut = upool.tile([S, 36], f32, tag="p2u")
                in1p = c32l[:, c * 36:(c + 1) * 36].rearrange("s (i k) -> s i k", i=K)
                in2p = (
                    ABf[:, c * K:(c + 1) * K].unsqueeze(1)
                    .broadcast_to([S, K, K])
                )
                uv = Ut[:].rearrange("s (i k) -> s i k", i=K)
                nc.vector.tensor_tensor(uv, in1p, in2p, op=ADD)
                nc.vector.tensor_reduce(
                    ABf[:, (c + 1) * K:(c + 2) * K], uv, axis=X, op=MAX
                )
            for c in range(NCH - 1, 0, -1):
                Ut = upool.tile([S, 36], f32, tag="p2u")
                in1p = (
                    c32l[:, c * 36:(c + 1) * 36]
                    .rearrange("s (i k) -> s i k", i=K).transpose([0, 2, 1])
                )
                in2p = (
                    BBw[:, c * K:(c + 1) * K].unsqueeze(1)
                    .broadcast_to([S, K, K])
                )
                uv = Ut[:].rearrange("s (k i) -> s k i", k=K)
                nc.vector.tensor_tensor(uv, in1p, in2p, op=ADD)
                nc.vector.tensor_reduce(
                    BBw[:, (c - 1) * K:c * K], uv, axis=X, op=MAX
                )

            # boundaries back to [(s,c)] rows
            A0 = spool.tile([ROWS, K], f32, tag="A0")
            B31 = spool.tile([ROWS, K], f32, tag="B31")
            nc.sync.dma_start(abstage_d[:], ABf[:])
            nc.sync.dma_start(bbstage_d[:], BBw[:])
            for s in range(S):
                nc.sync.dma_start(
                    A0[s * NCH:(s + 1) * NCH, :],
                    abstage_d[s:s + 1, :].rearrange("p (c i) -> p c i", c=NCH),
                )
                nc.sync.dma_start(
                    B31[s * NCH:(s + 1) * NCH, :],
                    bbstage_d[s:s + 1, :].rearrange("p (c i) -> p c i", c=NCH),
                )

            # pass 3: per-t alpha (AlphaX: n*33 + (j+1), jx=0 is the boundary)
            # and beta (BetaX: n*32 + j, j=31 holds the boundary value).
            AlphaX = spool.tile([ROWS, K * 33], f32, tag="AlphaX")
            BetaX = spool.tile([ROWS, K * SPAN], f32, tag="BetaX")
            AJ = AlphaX[:].rearrange("r (n j) -> r n j", j=33)
            BJ = BetaX[:].rearrange("r (n j) -> r n j", j=SPAN)
            nc.vector.tensor_copy(AJ[:, :, 0], A0[:])
            nc.vector.tensor_copy(BJ[:, :, 31], B31[:])

            c8v = C8_t[:].rearrange("r (g b) -> r g b", b=36)
            c4v = C4_t[:].rearrange("r (g b) -> r g b", b=36)
            c2v = C2_t[:].rearrange("r (g b) -> r g b", b=36)

            # ---- fwd ----
            # C8 chain: jx=8u -> jx=8u+8
            for u in range(4):
                Ut = upool.tile([ROWS, 36], f32, tag="p3u")
                in1p = c8v[:, u, :].rearrange("r (i k) -> r i k", i=K)
                in2p = AJ[:, :, 8 * u].unsqueeze(1).broadcast_to([ROWS, K, K])
                uv = Ut[:].rearrange("r (i k) -> r i k", i=K)
                nc.vector.tensor_tensor(uv, in1p, in2p, op=ADD)
                nc.vector.tensor_reduce(AJ[:, :, 8 * u + 8], uv, axis=X, op=MAX)
            # C4 batch: jx=8u -> jx=8u+4 via C4[2u]
            U4 = upool.tile([ROWS, 4 * 36], f32, tag="p3b")
            in1p = (
                c4v[:, 0:8:2, :].rearrange("r u (i k) -> r u i k", i=K)
            )
            in2p = (
                AJ[:, :, 0:25:8].transpose([0, 2, 1])
                .unsqueeze(2).broadcast_to([ROWS, 4, K, K])
            )
            uv = U4[:].rearrange("r (u i k) -> r u i k", u=4, i=K)
            nc.vector.tensor_tensor(uv, in1p, in2p, op=ADD)
            nc.vector.tensor_reduce(
                AJ[:, :, 4:29:8].transpose([0, 2, 1]), uv, axis=X, op=MAX
            )
            # C2 batch: jx=4v -> jx=4v+2 via C2[2v]
            U2 = upool.tile([ROWS, 8 * 36], f32, tag="p3b2")
            in1p = c2v[:, 0:16:2, :].rearrange("r v (i k) -> r v i k", i=K)
            in2p = (
                AJ[:, :, 0:29:4].transpose([0, 2, 1])
                .unsqueeze(2).broadcast_to([ROWS, 8, K, K])
            )
            uv = U2[:].rearrange("r (v i k) -> r v i k", v=8, i=K)
            nc.vector.tensor_tensor(uv, in1p, in2p, op=ADD)
            nc.vector.tensor_reduce(
                AJ[:, :, 2:31:4].transpose([0, 2, 1]), uv, axis=X, op=MAX
            )
            # evens: alpha[2u] = M_{2u} (x) alpha[2u-1]; jx=2u -> jx=2u+1
            UE = upool.tile([ROWS, 16 * 36], f32, tag="U")
            in1p = (
                transR_t[:].rearrange("r (i k) -> r i k", i=K)
                .unsqueeze(1).broadcast_to([ROWS, 16, K, K])
            )
            in2p = (
                AJ[:, :, 0:31:2].transpose([0, 2, 1])
                .unsqueeze(2).broadcast_to([ROWS, 16, K, K])
            )
            uev = UE[:].rearrange("r (u i k) -> r u i k", u=16, i=K)
            nc.vector.tensor_tensor(uev, in1p, in2p, op=ADD)
            VE = upool.tile([ROWS, 16 * K], f32, tag="V")
            vev = VE[:].rearrange("r (u i) -> r u i", u=16)
            nc.vector.tensor_reduce(vev, uev, axis=X, op=MAX)
            in2f = F3[:, :, 0:31:2].transpose([0, 2, 1])  # (u, i) = feat_{2u}[i]
            nc.vector.tensor_tensor(
                AJ[:, :, 1:32:2].transpose([0, 2, 1]), vev, in2f, op=ADD
            )

            # ---- bwd ----
            # C8 fold: j=8u+15 -> j=8u+7 via C8[u+1]
            for u in (2, 1, 0):
                Ut = upool.tile([ROWS, 36], f32, tag="p3u")
                in1p = (
                    c8v[:, u + 1, :].rearrange("r (i k) -> r i k", i=K)
                    .transpose([0, 2, 1])
                )
                in2p = BJ[:, :, 8 * u + 15].unsqueeze(1).broadcast_to([ROWS, K, K])
                uv = Ut[:].rearrange("r (k i) -> r k i", k=K)
                nc.vector.tensor_tensor(uv, in1p, in2p, op=ADD)
                nc.vector.tensor_reduce(BJ[:, :, 8 * u + 7], uv, axis=X, op=MAX)
            # C4 batch: j=8u+7 -> j=8u+3 via C4[2u+1]
            U4b = upool.tile([ROWS, 4 * 36], f32, tag="p3b")
            in1p = (
                c4v[:, 1:8:2, :].rearrange("r u (i k) -> r u i k", i=K)
                .transpose([0, 1, 3, 2])
            )
            in2p = (
                BJ[:, :, 7:32:8].transpose([0, 2, 1])
                .unsqueeze(2).broadcast_to([ROWS, 4, K, K])
            )
            uv = U4b[:].rearrange("r (u k i) -> r u k i", u=4, k=K)
            nc.vector.tensor_tensor(uv, in1p, in2p, op=ADD)
            nc.vector.tensor_reduce(
                BJ[:, :, 3:28:8].transpose([0, 2, 1]), uv, axis=X, op=MAX
            )
            # C2 batch: j=4v+3 -> j=4v+1 via C2[2v+1]
            U2b = upool.tile([ROWS, 8 * 36], f32, tag="p3b2")
            in1p = (
                c2v[:, 1:16:2, :].rearrange("r v (i k) -> r v i k", i=K)
                .transpose([0, 1, 3, 2])
            )
            in2p = (
                BJ[:, :, 3:32:4].transpose([0, 2, 1])
                .unsqueeze(2).broadcast_to([ROWS, 8, K, K])
            )
            uv = U2b[:].rearrange("r (v k i) -> r v k i", v=8, k=K)
            nc.vector.tensor_tensor(uv, in1p, in2p, op=ADD)
            nc.vector.tensor_reduce(
                BJ[:, :, 1:30:4].transpose([0, 2, 1]), uv, axis=X, op=MAX
            )
            # evens: beta[2u][k] = max_i(trans[i,k] + feat_{2u+1}[i] + beta[2u+1][i])
            W2 = upool.tile([ROWS, 16 * K], f32, tag="V")
            w2v = W2[:].rearrange("r (u i) -> r u i", u=16)
            nc.vector.tensor_tensor(
                w2v,
                F3[:, :, 1:32:2].transpose([0, 2, 1]),
                BJ[:, :, 1:32:2].transpose([0, 2, 1]),
                op=ADD,
            )
            UEb = upool.tile([ROWS, 16 * 36], f32, tag="U")
            in1p = (
                transR_t[:].rearrange("r (i k) -> r i k", i=K)
                .transpose([0, 2, 1])
                .unsqueeze(1).broadcast_to([ROWS, 16, K, K])
            )
            in2p = w2v.unsqueeze(2).broadcast_to([ROWS, 16, K, K])
            uebv = UEb[:].rearrange("r (u k i) -> r u k i", u=16, k=K)
            nc.vector.tensor_tensor(uebv, in1p, in2p, op=ADD)
            nc.vector.tensor_reduce(
                BJ[:, :, 0:31:2].transpose([0, 2, 1]), uebv, axis=X, op=MAX
            )

            # ---- extraction ----
            Sc = spool.tile([ROWS, K * SPAN], f32, tag="Sc")
            scv = Sc[:].rearrange("r (n j) -> r n j", n=K)
            nc.vector.tensor_tensor(scv, AJ[:, :, 1:33], BJ[:, :, :], op=ADD)
            m_t = spool.tile([ROWS, SPAN], f32, tag="m")
            nc.vector.tensor_reduce(
                m_t[:], scv.transpose([0, 2, 1]), axis=X, op=MAX
            )
            eq_t = upool.tile([ROWS, K * SPAN], f32, tag="U")
            eqv = eq_t[:].rearrange("r (n j) -> r n j", n=K)
            nc.vector.tensor_tensor(
                eqv, scv, m_t[:].unsqueeze(1).broadcast_to([ROWS, K, SPAN]), op=EQ
            )
            val_t = upool.tile([ROWS, K * SPAN], f32, tag="V")
            valv = val_t[:].rearrange("r (n j) -> r n j", n=K)
            nc.vector.tensor_tensor(
                valv, eqv, cn_t[:].rearrange("r (n j) -> r n j", n=K), op=MULT
            )
            q_t = spool.tile([ROWS, SPAN], f32, tag="q")
            nc.vector.tensor_reduce(
                q_t[:], valv.transpose([0, 2, 1]), axis=X, op=MAX
            )
            pathf_t = spool.tile([ROWS, SPAN], f32, tag="pathf")
            nc.vector.tensor_scalar(
                pathf_t[:], q_t[:], -1.0, float(K), op0=MULT, op1=ADD
            )
            pathi_t = spool.tile([ROWS, SPAN], i32, tag="pathi")
            nc.vector.tensor_copy(pathi_t[:], pathf_t[:])

            # ---- outputs ----
            nc.sync.dma_start(ps_out[:], m_t[0:ROWS:NCH, 0:1])
            for s in range(S):
                nc.sync.dma_start(
                    path_out[s:s + 1, :].rearrange("p (c j) -> p c j", c=NCH),
                    pathi_t[s * NCH:(s + 1) * NCH, :],
                )

    return _split_multiwaits(nc) if split else nc


def host_inputs(x, W, b, transitions):
    f32 = np.float32
    x = np.ascontiguousarray(np.asarray(x), dtype=f32)
    W = np.ascontiguousarray(np.asarray(W), dtype=f32)
    b = np.ascontiguousarray(np.asarray(b), dtype=f32)
    trans = np.ascontiguousarray(np.asarray(transitions), dtype=f32)

    wc = np.empty((128, HC * K), f32)
    for hc in range(HC):
        wc[:, hc * K:(hc + 1) * K] = W[hc * 128:(hc + 1) * 128, :]
    bvec = b.reshape(K, 1)
    transR = np.ascontiguousarray(
        np.broadcast_to(trans.reshape(1, 36), (128, 36))
    )
    t2 = (trans[:, None, :] + trans.T[None, :, :]).astype(f32)  # [i,p,k]
    t2r = np.ascontiguousarray(np.broadcast_to(t2.reshape(1, 216), (128, 216)))
    cnrow = np.repeat((K - np.arange(K)).astype(f32), SPAN)
    cn = np.ascontiguousarray(np.broadcast_to(cnrow[None, :], (128, K * SPAN)))
    a0row = np.full(K, NEG, f32)
    a0row[START_IDX] = 0.0
    a0 = np.ascontiguousarray(np.broadcast_to(a0row[None, :], (128, K)))
    ident = np.eye(128, dtype=f32)

    shared = {
        "wc": wc, "bvec": bvec, "transR": transR, "t2r": t2r,
        "cn": cn, "a0": a0, "ident": ident,
    }
    in_maps = [
        {"xs": np.ascontiguousarray(x[c * S:(c + 1) * S]), **shared}
        for c in range(NCORES)
    ]
    return in_maps


_NC_CACHE = {}


def kernel(x, W, b, transitions):
    from concourse.bass_utils import run_bass_kernel_spmd

    in_maps = host_inputs(x, W, b, transitions)
    if "nc" not in _NC_CACHE:
        _NC_CACHE["nc"] = build_module()
    nc = _NC_CACHE["nc"]
    res = run_bass_kernel_spmd(nc, in_maps, core_ids=list(range(NCORES)))
    ps = np.concatenate([res.results[c]["ps"][:, 0] for c in range(NCORES)])
    path = np.concatenate([res.results[c]["path"] for c in range(NCORES)])
    return ps.astype(np.float32), path.astype(np.int32)
